# revision 7
# baseline (speedup 1.0000x reference)
"""Trainium2 Bass kernel for nn_ACWAN (embedding + GRU + cosine-attention +
gated-state recurrence + output projection), data-parallel over batch on 8
NeuronCores.

Self-contained: hardcodes all shapes; host side only reformats weights/indices
and gathers per-core outputs.
"""
import sys

sys.path.insert(0, "/opt/trn_rl_repo")

import numpy as np
import ml_dtypes

import concourse.bass as bass
import concourse.mybir as mybir
from concourse.tile import TileContext
from concourse.bass_utils import run_bass_kernel_spmd
from concourse.masks import make_identity

# ---- problem dims (hardcoded) ----
B, T, E, H, NT, NCLS = 256, 512, 200, 100, 100000, 5
NCORES = 8
BS = B // NCORES            # 32 batch rows per core
TCH = 32                    # timesteps per chunk
NCH = T // TCH              # 16 chunks
SLOTS = BS * T              # 16384 gather slots per core (k = t*BS + b)
GT = SLOTS // 128           # 128 gather tiles (128 rows each)
GPC = GT // NCH             # 8 gather tiles per chunk

F32 = mybir.dt.float32
BF16 = mybir.dt.bfloat16
I32 = mybir.dt.int32
BF = ml_dtypes.bfloat16
AF = mybir.ActivationFunctionType
OP = mybir.AluOpType

MAX_WAITS_DEFAULT = 1


def _split_excess_waits(nc):
    """walrus here accepts very few sem-waits per instruction; hoist extras
    onto NoOps (1 wait each) placed just before, on the same engine."""
    n_fix = 0
    for f in nc.m.functions:
        for bb in f.blocks:
            out = []
            changed = False
            for ins in bb.instructions:
                si = ins.sync_info
                limit = MAX_WAITS_DEFAULT
                if si is not None and si.on_wait and len(si.on_wait) > limit:
                    waits = list(si.on_wait)
                    extra, keep = waits[:-limit], waits[-limit:]
                    for k, w in enumerate(extra):
                        out.append(
                            mybir.InstNoOp(
                                name=f"{ins.name}-wsplit{k}",
                                sync_info=mybir.SyncInfo(on_wait=[w], on_update=[]),
                                bass_nofuse=True,
                                engine=ins.engine,
                            )
                        )
                    ins.sync_info = mybir.SyncInfo(
                        on_wait=keep, on_update=list(si.on_update)
                    )
                    n_fix += 1
                    changed = True
                out.append(ins)
            if changed:
                bb.instructions = out
    return n_fix


def build_graph():
    nc = bass.Bass()

    dp = nc.declare_dram_parameter
    emb = dp("emb", [NT, E], F32, isOutput=False)
    idx = dp("idx", [128, GT], I32, isOutput=False)          # [p, tile]
    wih = dp("wih", [100, 600], BF16, isOutput=False)        # [e, (chunk,gate,m)]
    whh = dp("whh", [100, 300], F32, isOutput=False)        # [h, (gate,m)]
    biases = dp("biases", [100, 3], F32, isOutput=False)     # r,z fused; n = b_ih_n
    bhhn = dp("bhhn", [1, 100], F32, isOutput=False)
    wti = dp("wti", [100, 100], BF16, isOutput=False)
    bti = dp("bti", [100, 1], F32, isOutput=False)           # b_ti + b_ts fused
    wls = dp("wls", [100, 1], F32, isOutput=False)
    wli = dp("wli", [100, 1], BF16, isOutput=False)
    blgr = dp("blgr", [1, 1], F32, isOutput=False)
    wts = dp("wts", [100, 100], F32, isOutput=False)
    wout = dp("wout", [100, NCLS], F32, isOutput=False)
    bout = dp("bout", [NCLS, 1], F32, isOutput=False)
    lens = dp("lens", [1, BS], F32, isOutput=False)
    tvals = dp("tvals", [8, 128], F32, isOutput=False)       # p*4 + f//32
    out_d = dp("out", [NCLS, BS], F32, isOutput=True)

    with TileContext(nc) as tc:
        with tc.tile_pool(name="const", bufs=1) as cp, \
             tc.tile_pool(name="big", bufs=1) as bigp, \
             tc.tile_pool(name="xpring", bufs=3) as xpp, \
             tc.tile_pool(name="gring", bufs=4) as gp, \
             tc.tile_pool(name="rtring", bufs=3) as rtp, \
             tc.tile_pool(name="s1sc", bufs=3) as s1p, \
             tc.tile_pool(name="s2sc", bufs=3) as s2p, \
             tc.tile_pool(name="blksc", bufs=2) as bkp, \
             tc.tile_pool(name="pa", bufs=3, space="PSUM") as pa, \
             tc.tile_pool(name="p1", bufs=2, space="PSUM") as p1, \
             tc.tile_pool(name="p2", bufs=2, space="PSUM") as p2, \
             tc.tile_pool(name="pb", bufs=1, space="PSUM") as pb:

            # ---------- load constants ----------
            idx_sb = cp.tile([128, GT], I32)
            nc.sync.dma_start(out=idx_sb[:], in_=idx[:])
            wih_sb = cp.tile([100, 600], BF16)
            nc.sync.dma_start(out=wih_sb[:], in_=wih[:])
            whh_sb = cp.tile([100, 300], F32)
            nc.sync.dma_start(out=whh_sb[:], in_=whh[:])
            bias_sb = cp.tile([100, 3], F32)
            nc.sync.dma_start(out=bias_sb[:], in_=biases[:])
            bhhn_sb = cp.tile([1, 100], F32)
            nc.sync.dma_start(out=bhhn_sb[:], in_=bhhn[:])
            wti_sb = cp.tile([100, 100], BF16)
            nc.sync.dma_start(out=wti_sb[:], in_=wti[:])
            bti_sb = cp.tile([100, 1], F32)
            nc.sync.dma_start(out=bti_sb[:], in_=bti[:])
            wls_sb = cp.tile([100, 1], F32)
            nc.sync.dma_start(out=wls_sb[:], in_=wls[:])
            wli_sb = cp.tile([100, 1], BF16)
            nc.sync.dma_start(out=wli_sb[:], in_=wli[:])
            blgr_sb = cp.tile([1, 1], F32)
            nc.sync.dma_start(out=blgr_sb[:], in_=blgr[:])
            wts_sb = cp.tile([100, 100], F32)
            nc.sync.dma_start(out=wts_sb[:], in_=wts[:])
            wout_sb = cp.tile([100, NCLS], F32)
            nc.sync.dma_start(out=wout_sb[:], in_=wout[:])
            bout_sb = cp.tile([NCLS, 1], F32)
            nc.sync.dma_start(out=bout_sb[:], in_=bout[:])
            tvals_sb = cp.tile([8, 128], F32)
            nc.sync.dma_start(out=tvals_sb[:], in_=tvals[:])

            lens_sb = cp.tile([8, 128], F32)
            lens_bcast = bass.AP(
                tensor=lens[:].tensor, offset=0,
                ap=[[0, 8], [0, 4], [1, BS]],
            )
            nc.sync.dma_start(
                out=lens_sb[:].rearrange("p (a b) -> p a b", a=4), in_=lens_bcast
            )

            ident = cp.tile([128, 128], BF16)
            make_identity(nc, ident[:])
            ones32 = cp.tile([1, 32], F32)
            nc.vector.memset(ones32[:], 1.0)
            ones100 = cp.tile([1, 100], BF16)
            nc.vector.memset(ones100[:], 1.0)
            ones_col = cp.tile([100, 1], BF16)
            nc.vector.memset(ones_col[:], 1.0)
            h0 = cp.tile([100, BS], F32)
            nc.vector.memset(h0[:], 0.0)
            st0 = cp.tile([100, BS], F32)
            nc.vector.memset(st0[:], 0.0)
            eps24 = cp.tile([8, 1], F32)
            nc.vector.memset(eps24[:], 1e-24)

            # persistent rows (free-dim offset space is the scarce resource)
            li_row = bigp.tile([1, SLOTS], BF16)     # lgr input + b_lgr
            z_row = bigp.tile([1, SLOTS], BF16)      # relu(att)*mask
            rnn = [bigp.tile([100, TCH * BS], BF16, tag=f"rnn{c}", name=f"rnn{c}") for c in range(NCH)]
            tis = [bigp.tile([100, TCH * BS], BF16, tag=f"ti{c}", name=f"ti{c}") for c in range(NCH)]

            # ---------- stage emitters ----------
            def emit_gather_chunk(c):
                xpc = xpp.tile([100, TCH, 3, BS], BF16, tag="xp")
                for j in range(GPC):
                    tile_id = c * GPC + j
                    g = gp.tile([128, E], BF16, tag="g")
                    nc.gpsimd.indirect_dma_start(
                        out=g[:], out_offset=None, in_=emb[:],
                        in_offset=bass.IndirectOffsetOnAxis(
                            ap=idx_sb[:, tile_id:tile_id + 1], axis=0),
                    )
                    rt = rtp.tile([100, 256], BF16, tag="rt")
                    for ch in range(2):
                        tr = pa.tile([100, 128], BF16, space="PSUM", tag="pa")
                        nc.tensor.transpose(
                            out=tr[:], in_=g[:, ch * 100:(ch + 1) * 100],
                            identity=ident[:])
                        if ch == 0:
                            nc.vector.tensor_copy(
                                out=rt[:, ch * 128:(ch + 1) * 128], in_=tr[:])
                        else:
                            nc.scalar.copy(
                                out=rt[:, ch * 128:(ch + 1) * 128], in_=tr[:])
                    for gate in range(3):
                        xg = pa.tile([100, 128], F32, space="PSUM", tag="pa")
                        nc.tensor.matmul(
                            out=xg[:], lhsT=wih_sb[:, (0 * 3 + gate) * 100:(0 * 3 + gate) * 100 + 100],
                            rhs=rt[:, 0:128], start=True, stop=False)
                        nc.tensor.matmul(
                            out=xg[:], lhsT=wih_sb[:, (1 * 3 + gate) * 100:(1 * 3 + gate) * 100 + 100],
                            rhs=rt[:, 128:256], start=False, stop=True)
                        # out slice (100, 4, 32) strided into xp chunk + bias
                        dst = xpc[:, 4 * j:4 * j + 4, gate, :]
                        src = xg[:].rearrange("p (t b) -> p t b", t=4)
                        nc.vector.tensor_scalar_add(
                            out=dst, in0=src, scalar1=bias_sb[:, gate:gate + 1])
                return xpc

            def emit_scan1_chunk(c, xpc, h_prev):
                for tl in range(TCH):
                    t = c * TCH + tl
                    P = p1.tile([100, 96], F32, space="PSUM", tag="p1")
                    nc.tensor.matmul(out=P[:, 64:96], lhsT=bhhn_sb[:],
                                     rhs=ones32[:], start=True, stop=False)
                    nc.tensor.matmul(out=P[:, 64:96], lhsT=whh_sb[:, 200:300],
                                     rhs=h_prev, start=False, stop=True)
                    nc.tensor.matmul(out=P[:, 0:32], lhsT=whh_sb[:, 0:100],
                                     rhs=h_prev, start=True, stop=True)
                    nc.tensor.matmul(out=P[:, 32:64], lhsT=whh_sb[:, 100:200],
                                     rhs=h_prev, start=True, stop=True)
                    A = s1p.tile([100, 64], F32, tag="A")
                    nc.vector.tensor_tensor(
                        out=A[:], in0=P[:, 0:64],
                        in1=xpc[:, tl, 0:2, :], op=OP.add)
                    S = s1p.tile([100, 64], BF16, tag="S")
                    nc.scalar.activation(out=S[:], in_=A[:], func=AF.Sigmoid)
                    u = s1p.tile([100, 32], BF16, tag="u")
                    nc.scalar.activation(out=u[:], in_=A[:, 32:64],
                                         func=AF.Sigmoid, scale=-1.0)
                    m2 = s1p.tile([100, 32], F32, tag="m2")
                    nc.vector.tensor_tensor(out=m2[:], in0=S[:, 32:64],
                                            in1=h_prev, op=OP.mult)
                    t1 = s1p.tile([100, 32], F32, tag="t1")
                    nc.vector.tensor_tensor(out=t1[:], in0=S[:, 0:32],
                                            in1=P[:, 64:96], op=OP.mult)
                    t2 = s1p.tile([100, 32], F32, tag="t2")
                    nc.vector.tensor_tensor(out=t2[:], in0=t1[:],
                                            in1=xpc[:, tl, 2, :], op=OP.add)
                    ng = s1p.tile([100, 32], BF16, tag="ng")
                    nc.scalar.activation(out=ng[:], in_=t2[:], func=AF.Tanh)
                    m1 = s1p.tile([100, 32], F32, tag="m1")
                    nc.vector.tensor_tensor(out=m1[:], in0=u[:], in1=ng[:],
                                            op=OP.mult)
                    hf = s1p.tile([100, 32], F32, tag="hf")
                    nc.vector.tensor_tensor(out=hf[:], in0=m1[:], in1=m2[:],
                                            op=OP.add)
                    nc.scalar.copy(out=rnn[c][:, tl * BS:(tl + 1) * BS],
                                   in_=hf[:])
                    h_prev = hf[:]
                return h_prev

            def emit_bulk(c):
                R = rnn[c]
                # ti (+ fused b_ti + b_ts)
                for hh in range(2):
                    pt = pb.tile([100, 512], F32, space="PSUM", tag="pb")
                    nc.tensor.matmul(out=pt[:], lhsT=wti_sb[:],
                                     rhs=R[:, hh * 512:(hh + 1) * 512],
                                     start=True, stop=True)
                    nc.vector.tensor_scalar_add(
                        out=tis[c][:, hh * 512:(hh + 1) * 512], in0=pt[:],
                        scalar1=bti_sb[:])
                # li row (+ b_lgr)
                for hh in range(2):
                    pl = pb.tile([1, 512], F32, space="PSUM", tag="pb")
                    nc.tensor.matmul(out=pl[:], lhsT=wli_sb[:],
                                     rhs=R[:, hh * 512:(hh + 1) * 512],
                                     start=True, stop=True)
                    nc.scalar.activation(
                        out=li_row[0:1, c * 1024 + hh * 512:c * 1024 + (hh + 1) * 512],
                        in_=pl[:], func=AF.Identity, bias=blgr_sb[:])
                # s1/s2 rows
                sq = bkp.tile([100, TCH * BS], BF16, tag="sq")
                nc.scalar.activation(out=sq[:], in_=R[:], func=AF.Square)
                s12 = bkp.tile([1, 2048], F32, tag="s12")
                for hh in range(2):
                    ps = pb.tile([1, 512], F32, space="PSUM", tag="pb")
                    nc.tensor.matmul(out=ps[:], lhsT=ones_col[:],
                                     rhs=R[:, hh * 512:(hh + 1) * 512],
                                     start=True, stop=True)
                    nc.vector.tensor_copy(out=s12[0:1, hh * 512:(hh + 1) * 512],
                                          in_=ps[:])
                for hh in range(2):
                    ps = pb.tile([1, 512], F32, space="PSUM", tag="pb")
                    nc.tensor.matmul(out=ps[:], lhsT=ones_col[:],
                                     rhs=sq[:, hh * 512:(hh + 1) * 512],
                                     start=True, stop=True)
                    nc.vector.tensor_copy(out=s12[0:1, 1024 + hh * 512:1024 + (hh + 1) * 512],
                                          in_=ps[:])
                # reshape rows to (8,128) for cheap row math
                s1w = bkp.tile([8, 128], F32, tag="s1w")
                s2w = bkp.tile([8, 128], F32, tag="s2w")
                nc.sync.dma_start(out=s1w[:], in_=s12[0:1, 0:1024])
                nc.sync.dma_start(out=s2w[:], in_=s12[0:1, 1024:2048])
                nrm = bkp.tile([8, 128], F32, tag="nrm")
                nc.scalar.activation(out=nrm[:], in_=s2w[:], func=AF.Sqrt,
                                     bias=eps24[:])
                rcp = bkp.tile([8, 128], F32, tag="rcp")
                nc.vector.reciprocal(out=rcp[:], in_=nrm[:])
                att = bkp.tile([8, 128], F32, tag="att")
                nc.vector.tensor_tensor(out=att[:], in0=s1w[:], in1=rcp[:],
                                        op=OP.mult)
                z1 = bkp.tile([8, 128], F32, tag="z1")
                nc.vector.tensor_scalar(out=z1[:], in0=att[:], scalar1=0.0,
                                        scalar2=1e-3, op0=OP.max, op1=OP.mult)
                cmp = bkp.tile([8, 128], F32, tag="cmp")
                nc.vector.scalar_tensor_tensor(
                    out=cmp[:], in0=tvals_sb[:], scalar=float(c * TCH),
                    in1=lens_sb[:], op0=OP.add, op1=OP.is_lt)
                zw = bkp.tile([8, 128], BF16, tag="zw")
                nc.vector.tensor_tensor(out=zw[:], in0=z1[:], in1=cmp[:],
                                        op=OP.mult)
                nc.sync.dma_start(
                    out=z_row[0:1, c * 1024:(c + 1) * 1024], in_=zw[:])

            def emit_scan2_chunk(c, st_prev):
                for tl in range(TCH):
                    t = c * TCH + tl
                    T2 = p2.tile([100, 128], F32, space="PSUM", tag="p2")
                    nc.tensor.matmul(out=T2[:, 0:32], lhsT=wts_sb[:],
                                     rhs=st_prev, start=True, stop=True)
                    nc.tensor.matmul(out=T2[0:1, 32:64], lhsT=wls_sb[:],
                                     rhs=st_prev, start=True, stop=True)
                    nc.tensor.matmul(out=T2[:, 96:128], lhsT=ones100[:],
                                     rhs=z_row[0:1, t * BS:(t + 1) * BS],
                                     start=True, stop=True)
                    gin = s2p.tile([1, 32], F32, tag="gin")
                    nc.vector.tensor_tensor(
                        out=gin[:], in0=T2[0:1, 32:64],
                        in1=li_row[0:1, t * BS:(t + 1) * BS], op=OP.add)
                    gt = s2p.tile([1, 32], BF16, tag="gt")
                    nc.scalar.activation(out=gt[:], in_=gin[:], func=AF.Sigmoid)
                    nc.tensor.matmul(out=T2[:, 64:96], lhsT=ones100[:],
                                     rhs=gt[:], start=True, stop=True)
                    cts = s2p.tile([100, 32], BF16, tag="cts")
                    nc.scalar.copy(out=cts[:], in_=T2[:, 0:32])
                    m2 = s2p.tile([100, 32], F32, tag="m2b")
                    nc.vector.tensor_tensor(out=m2[:], in0=T2[:, 96:128],
                                            in1=st_prev, op=OP.mult)
                    d = s2p.tile([100, 32], F32, tag="d")
                    nc.vector.tensor_tensor(out=d[:], in0=st_prev, in1=m2[:],
                                            op=OP.subtract)
                    t3 = s2p.tile([100, 32], F32, tag="t3")
                    nc.vector.tensor_tensor(out=t3[:], in0=T2[:, 64:96],
                                            in1=cts[:], op=OP.mult)
                    t4 = s2p.tile([100, 32], F32, tag="t4")
                    nc.vector.tensor_tensor(
                        out=t4[:], in0=t3[:],
                        in1=tis[c][:, tl * BS:(tl + 1) * BS], op=OP.add)
                    ns = s2p.tile([100, 32], BF16, tag="ns")
                    nc.scalar.activation(out=ns[:], in_=t4[:], func=AF.Tanh)
                    m1 = s2p.tile([100, 32], F32, tag="m1b")
                    nc.vector.tensor_tensor(out=m1[:], in0=T2[:, 96:128],
                                            in1=ns[:], op=OP.mult)
                    st_new = s2p.tile([100, BS], F32, tag="st")
                    nc.vector.tensor_tensor(out=st_new[:], in0=d[:], in1=m1[:],
                                            op=OP.add)
                    st_prev = st_new[:]
                return st_prev

            # ---------- emit pipeline ----------
            h_prev = h0[:]
            st_prev = st0[:]
            for c in range(NCH):
                xpc = emit_gather_chunk(c)
                h_prev = emit_scan1_chunk(c, xpc, h_prev)
                if c >= 1:
                    st_prev = emit_scan2_chunk(c - 1, st_prev)
                emit_bulk(c)
            st_prev = emit_scan2_chunk(NCH - 1, st_prev)

            # ---------- output ----------
            po = p2.tile([100, 128], F32, space="PSUM", tag="p2")
            nc.tensor.matmul(out=po[0:NCLS, 0:BS], lhsT=wout_sb[:],
                             rhs=st_prev, start=True, stop=True)
            osb = s2p.tile([NCLS, BS], F32, tag="osb")
            nc.scalar.activation(out=osb[:], in_=po[0:NCLS, 0:BS],
                                 func=AF.Identity, bias=bout_sb[:])
            nc.sync.dma_start(out=out_d[:], in_=osb[:])

    _split_excess_waits(nc)
    return nc


_NC = None


def _get_nc():
    global _NC
    if _NC is None:
        _NC = build_graph()
    return _NC


def _prep_core_inputs(txt_s, lens_s, shared):
    """Per-core host prep: gather indices + lens."""
    flat = np.ascontiguousarray(txt_s.T).reshape(-1)  # slot k = t*BS + b
    idx_p = np.ascontiguousarray(
        flat.reshape(GT, 128).T).astype(np.int32)      # [p, tile]
    lens_p = lens_s.astype(np.float32).reshape(1, BS)
    m = dict(shared)
    m["idx"] = idx_p
    m["lens"] = lens_p
    return m


def _prep_shared(emb, W_ih, W_hh, b_ih, b_hh, W_lgr, b_lgr, W_ts, b_ts,
                 W_ti, b_ti, W_out, b_out):
    f32 = np.float32
    emb = np.ascontiguousarray(emb, dtype=f32)
    Wg = np.asarray(W_ih, f32).reshape(3, H, E)        # [g, m, e]
    arr = Wg.transpose(2, 0, 1)                        # [e, g, m]
    wih_p = np.ascontiguousarray(
        np.stack([arr[0:100], arr[100:200]], axis=1).reshape(100, 600)
    ).astype(BF)
    Whg = np.asarray(W_hh, f32).reshape(3, H, H)       # [g, m, h]
    whh_p = np.ascontiguousarray(
        Whg.transpose(2, 0, 1).reshape(H, 300)).astype(f32)
    b_ih = np.asarray(b_ih, f32)
    b_hh = np.asarray(b_hh, f32)
    biases_p = np.stack(
        [b_ih[0:H] + b_hh[0:H], b_ih[H:2 * H] + b_hh[H:2 * H], b_ih[2 * H:]],
        axis=1).astype(f32)                            # (100, 3)
    bhhn_p = b_hh[2 * H:].reshape(1, H).astype(f32)
    wti_p = np.ascontiguousarray(np.asarray(W_ti, f32).T).astype(BF)
    bti_p = (np.asarray(b_ti, f32) + np.asarray(b_ts, f32)).reshape(H, 1)
    W_lgr = np.asarray(W_lgr, f32)
    wls_p = np.ascontiguousarray(W_lgr[0, H:].reshape(H, 1)).astype(f32)
    wli_p = np.ascontiguousarray(W_lgr[0, :H].reshape(H, 1)).astype(BF)
    blgr_p = np.asarray(b_lgr, f32).reshape(1, 1)
    wts_p = np.ascontiguousarray(np.asarray(W_ts, f32).T).astype(f32)
    wout_p = np.ascontiguousarray(np.asarray(W_out, f32).T).astype(f32)
    bout_p = np.asarray(b_out, f32).reshape(NCLS, 1)
    tv = (np.arange(8)[:, None] * 4 + np.arange(128)[None, :] // 32).astype(f32)
    return {
        "emb": emb, "wih": wih_p, "whh": whh_p, "biases": biases_p,
        "bhhn": bhhn_p, "wti": wti_p, "bti": bti_p, "wls": wls_p,
        "wli": wli_p, "blgr": blgr_p, "wts": wts_p, "wout": wout_p,
        "bout": bout_p, "tvals": tv,
    }


def run(inputs, trace=False):
    txt = np.asarray(inputs["txt"]).astype(np.int32)
    lens = np.asarray(inputs["lens"]).astype(np.int32)
    shared = _prep_shared(
        inputs["emb"], inputs["W_ih"], inputs["W_hh"], inputs["b_ih"],
        inputs["b_hh"], inputs["W_lgr"], inputs["b_lgr"], inputs["W_ts"],
        inputs["b_ts"], inputs["W_ti"], inputs["b_ti"], inputs["W_out"],
        inputs["b_out"])
    in_maps = []
    for core in range(NCORES):
        sl = slice(core * BS, (core + 1) * BS)
        in_maps.append(_prep_core_inputs(txt[sl], lens[sl], shared))
    nc = _get_nc()
    res = run_bass_kernel_spmd(nc, in_maps, core_ids=list(range(NCORES)),
                               trace=trace)
    out = np.empty((B, NCLS), np.float32)
    for core in range(NCORES):
        out[core * BS:(core + 1) * BS] = res.results[core]["out"].T
    return out, res.exec_time_ns


def kernel(**inputs) -> np.ndarray:
    out, _ = run(inputs, trace=False)
    return out
